# revision 5
# baseline (speedup 1.0000x reference)
"""ABMIL (attention-based MIL) Trainium2 kernel, 8-core data-parallel.

Shapes (hardcoded): B=8 bags, N=1024 instances, D=1024, H=16 heads, hd=64.
Each NeuronCore processes one bag. Parameters are replicated; all big
tensors are host-packed into [128, F] layouts so every DMA is 128
contiguous descriptors (one per partition).

Math (rank-1 attention: only the cls query row survives):
  znorm      = (z - mu) * rsqrt(var + eps)          rows of z = [cls; x_b]
  ln0        = znorm0 * gamma + beta
  q          = w_q @ ln0 + b_q
  U[h]       = w_k[64h:64h+64].T @ q[64h:64h+64]            (16 x 1024)
  Ug         = U * gamma * 0.125 ;  e_h = 0.125*(U[h]@beta + q_h@b_k_h)
  scores     = Ug @ znorm.T + e                              (16 x 1025)
  A          = softmax(scores)
  Mrow       = (A @ znorm) * gamma + beta                    (16 x 1024)
  ctx[64h:+64] = w_v[64h:+64] @ Mrow[h] + b_v[64h:+64]
  out        = w_o @ ctx + b_o
"""

import numpy as np
import ml_dtypes

import concourse.bass as bass
import concourse.bacc as bacc
import concourse.mybir as mybir
import concourse.tile as tile
from concourse.bass_utils import run_bass_kernel_spmd

F32 = mybir.dt.float32
BF16 = mybir.dt.bfloat16
AX = mybir.AxisListType.X
OP = mybir.AluOpType
AF = mybir.ActivationFunctionType

D = 1024
NK = 8          # number of 128-chunks of D (and of x rows)
H = 16
EPS = 1e-5

_CACHE = {}


def _build():
    nc = bacc.Bacc()

    # Host-packed inputs: every [128, F] tensor DMAs as 128 contiguous rows.
    x_e = nc.declare_dram_parameter("x", [128, NK * D], BF16, isOutput=False)
    wq_e = nc.declare_dram_parameter("wqp", [128, NK * D], BF16, isOutput=False)
    wk_e = nc.declare_dram_parameter("wkp", [128, NK * D], BF16, isOutput=False)
    wv_e = nc.declare_dram_parameter("wvp", [128, NK * D], BF16, isOutput=False)
    wo_e = nc.declare_dram_parameter("wop", [128, NK * D], BF16, isOutput=False)
    # smalls: [gam_col, bet_col, bq_col, bv_col] each [128, 8] f32
    sm_e = nc.declare_dram_parameter("smalls", [128, 4 * NK], F32, isOutput=False)
    bk_e = nc.declare_dram_parameter("bkc", [128, NK], BF16, isOutput=False)
    cls_e = nc.declare_dram_parameter("cls", [D], F32, isOutput=False)
    gam_e = nc.declare_dram_parameter("gam", [D], F32, isOutput=False)
    bet_e = nc.declare_dram_parameter("bet", [D], F32, isOutput=False)
    bo_e = nc.declare_dram_parameter("bo", [D], BF16, isOutput=False)
    out_e = nc.declare_dram_parameter("out", [1, D], F32, isOutput=True)

    with tile.TileContext(nc) as tc:
        with (
            tc.tile_pool(name="singles", bufs=1) as singles,
            tc.tile_pool(name="xin", bufs=8) as xin,
            tc.tile_pool(name="work", bufs=4) as work,
            tc.tile_pool(name="pt", bufs=2, space="PSUM") as pt,
            tc.tile_pool(name="pbig", bufs=2, space="PSUM") as pbig,
            tc.tile_pool(name="psm", bufs=2, space="PSUM") as psm,
        ):
            # ---- small loads first (scalar ring), then big weights ------
            ident = singles.tile([128, 128], BF16, tag="ident")
            ident_dram = nc.inline_tensor(
                np.eye(128, dtype=ml_dtypes.bfloat16), name="ident_const"
            )

            eps_t = singles.tile([128, 1], F32, tag="eps")
            nc.vector.memset(eps_t[:, :], EPS)

            cls_row = singles.tile([1, D], F32, tag="clsr")
            sm = singles.tile([128, 4 * NK], F32, tag="smalls")
            bk_col = singles.tile([128, NK], BF16, tag="bkc")
            bo_row = singles.tile([1, D], BF16, tag="bor")
            gam_col = sm[:, 0 * NK : 1 * NK]
            bet_col = sm[:, 1 * NK : 2 * NK]
            bq_col = sm[:, 2 * NK : 3 * NK]
            bv_col = sm[:, 3 * NK : 4 * NK]

            # big persistent tiles
            wq_all = singles.tile([128, NK * D], BF16, tag="wq")
            wk_all = singles.tile([128, NK * D], BF16, tag="wk")
            wv_all = singles.tile([128, NK * D], BF16, tag="wv")
            wo_all = singles.tile([128, NK * D], BF16, tag="wo")
            znorm_all = singles.tile([128, NK * D], BF16, tag="znorm")
            znT_all = singles.tile([128, NK * D], BF16, tag="znT")
            znT3 = znT_all[:, :].rearrange("p (c s) -> p c s", c=NK)

            xks = [xin.tile([128, D], BF16, tag="xk", name=f"xk{i}") for i in range(NK)]

            # --- DMA schedule -------------------------------------------
            # sync ring:   cls, x0..x7, wk, wv   (+ even transposes below)
            # scalar ring: smalls, bk, bo, ident, wq, wo (+ odd transposes)
            nc.sync.dma_start(out=cls_row[:, :], in_=cls_e[None, :])
            nc.scalar.dma_start(out=sm[:, :], in_=sm_e[:, :])
            nc.scalar.dma_start(out=bk_col[:, :], in_=bk_e[:, :])
            nc.scalar.dma_start(out=bo_row[:, :], in_=bo_e[None, :])
            nc.scalar.dma_start(out=ident[:, :], in_=ident_dram[:, :])
            for k in range(NK):
                nc.sync.dma_start(out=xks[k][:, :], in_=x_e[:, D * k : D * (k + 1)])
            nc.scalar.dma_start(out=wq_all[:, :], in_=wq_e[:, :])
            nc.sync.dma_start(out=wk_all[:, :], in_=wk_e[:, :])
            nc.sync.dma_start(out=wv_all[:, :], in_=wv_e[:, :])
            nc.scalar.dma_start(out=wo_all[:, :], in_=wo_e[:, :])

            gam16 = singles.tile([H, D], F32, tag="gam16")
            nc.gpsimd.dma_start(
                out=gam16[:, :],
                in_=bass.AP(tensor=gam_e[:].tensor, offset=0, ap=[[0, H], [1, D]]),
            )
            bet16 = singles.tile([H, D], F32, tag="bet16")
            nc.gpsimd.dma_start(
                out=bet16[:, :],
                in_=bass.AP(tensor=bet_e[:].tensor, offset=0, ap=[[0, H], [1, D]]),
            )

            # ---- cls row LN (first DVE work: critical chain) -----------
            stats0 = work.tile([1, 2, 6], F32, tag="stats0")
            nc.vector.bn_stats(out=stats0[:, 0, :], in_=cls_row[:, 0:512])
            nc.vector.bn_stats(out=stats0[:, 1, :], in_=cls_row[:, 512:1024])
            mv0 = work.tile([1, 2], F32, tag="mv0")
            nc.vector.bn_aggr(out=mv0[:, :], in_=stats0[:, :, :])
            nc.scalar.activation(
                out=mv0[:, 1:2], in_=mv0[:, 1:2], func=AF.Sqrt,
                bias=eps_t[0:1, :], scale=1.0,
            )
            rs0 = work.tile([1, 1], F32, tag="rs0")
            nc.vector.tensor_copy(out=rs0[:, :], in_=mv0[:, 1:2])
            nc.vector.reciprocal(out=rs0[:, :], in_=rs0[:, :])
            zn0_row = singles.tile([1, D], BF16, tag="zn0r")
            nc.vector.tensor_scalar(
                out=zn0_row[:, :], in0=cls_row[:, :],
                scalar1=mv0[:, 0:1], scalar2=rs0[:, 0:1],
                op0=OP.subtract, op1=OP.mult,
            )

            # ---- znorm0 column layout + q (gamma/beta applied in cols) --
            lzp = pt.tile([128, 16], BF16, tag="pt")
            for c in range(NK):
                nc.tensor.transpose(
                    out=lzp[:, 2 * c : 2 * c + 1],
                    in_=zn0_row[0:1, 128 * c : 128 * (c + 1)],
                    identity=ident[0:1, 0:1],
                )
            zn0_col = singles.tile([128, NK], BF16, tag="zn0c")
            nc.scalar.copy(
                out=zn0_col[:, :],
                in_=lzp[:, :].rearrange("p (c x) -> p c x", c=NK)[:, :, 0],
            )
            ln0_col = singles.tile([128, NK], BF16, tag="ln0c")
            nc.vector.tensor_mul(out=ln0_col[:, :], in0=zn0_col[:, :], in1=gam_col[:, :])
            nc.vector.tensor_add(out=ln0_col[:, :], in0=ln0_col[:, :], in1=bet_col[:, :])

            psq = pbig.tile([1, D], F32, tag="pbig")
            for c in range(NK):
                for half in range(2):
                    nc.tensor.matmul(
                        psq[:, 512 * half : 512 * (half + 1)], lhsT=ln0_col[:, c : c + 1],
                        rhs=wq_all[:, D * c + 512 * half : D * c + 512 * (half + 1)],
                        start=(c == 0), stop=(c == NK - 1),
                        skip_group_check=True,
                    )
            q_sb = singles.tile([1, D], BF16, tag="qsb")
            nc.scalar.copy(out=q_sb[:, :], in_=psq[:, :])

            qcp = pt.tile([128, 16], BF16, tag="pt")
            for c in range(NK):
                nc.tensor.transpose(
                    out=qcp[:, 2 * c : 2 * c + 1],
                    in_=q_sb[0:1, 128 * c : 128 * (c + 1)],
                    identity=ident[0:1, 0:1],
                )
            q_col = singles.tile([128, NK], BF16, tag="qcol")
            nc.scalar.copy(
                out=q_col[:, :],
                in_=qcp[:, :].rearrange("p (c x) -> p c x", c=NK)[:, :, 0],
            )
            nc.vector.tensor_add(out=q_col[:, :], in0=q_col[:, :], in1=bq_col[:, :])
            qbT = singles.tile([128, H * NK], BF16, tag="qbT")
            nc.gpsimd.memset(qbT[:, :], 0.0)
            for c in range(NK):
                nc.gpsimd.tensor_copy(
                    out=qbT[0:64, H * c + 2 * c : H * c + 2 * c + 1],
                    in_=q_col[0:64, c : c + 1],
                )
                nc.gpsimd.tensor_copy(
                    out=qbT[64:128, H * c + 2 * c + 1 : H * c + 2 * c + 2],
                    in_=q_col[64:128, c : c + 1],
                )

            # ---- U = Qblk @ w_k ; Ug, e --------------------------------
            psU = pbig.tile([H, D], F32, tag="pbig")
            for c in range(NK):
                for half in range(2):
                    nc.tensor.matmul(
                        psU[:, 512 * half : 512 * (half + 1)], lhsT=qbT[:, H * c : H * (c + 1)],
                        rhs=wk_all[:, D * c + 512 * half : D * c + 512 * (half + 1)],
                        start=(c == 0), stop=(c == NK - 1),
                        skip_group_check=True,
                    )
            ug = singles.tile([H, D], BF16, tag="ug")
            nc.vector.scalar_tensor_tensor(
                out=ug[:, :], in0=psU[:, :], scalar=0.125, in1=gam16[:, :],
                op0=OP.mult, op1=OP.mult,
            )
            tmp16 = work.tile([H, D], F32, tag="tmp16")
            nc.vector.tensor_mul(out=tmp16[:, :], in0=psU[:, :], in1=bet16[:, :])
            e1 = work.tile([H, 1], F32, tag="e1")
            nc.vector.reduce_sum(out=e1[:, :], in_=tmp16[:, :], axis=AX)
            pse2 = psm.tile([H, 1], F32, tag="psm")
            for c in range(NK):
                nc.tensor.matmul(
                    pse2[:, :], lhsT=qbT[:, H * c : H * (c + 1)], rhs=bk_col[:, c : c + 1],
                    start=(c == 0), stop=(c == NK - 1),
                )
            e_sb = singles.tile([H, 1], F32, tag="esb")
            nc.vector.tensor_add(out=e_sb[:, :], in0=e1[:, :], in1=pse2[:, :])
            nc.vector.tensor_scalar_mul(out=e_sb[:, :], in0=e_sb[:, :], scalar1=0.125)

            ugp = pt.tile([128, 128], BF16, tag="pt")
            for c in range(NK):
                nc.tensor.transpose(
                    out=ugp[:, H * c : H * (c + 1)], in_=ug[:, 128 * c : 128 * (c + 1)],
                    identity=ident[0:H, 0:H],
                )
            ugT = singles.tile([128, H * NK], BF16, tag="ugT")
            nc.scalar.copy(out=ugT[:, :], in_=ugp[:, :])

            # per-head safe softmax shift: bound_h = 8*||Ug_h|| >= max score
            u2 = work.tile([H, D], F32, tag="u2")
            nc.vector.tensor_mul(out=u2[:, :], in0=ug[:, :], in1=ug[:, :])
            s2 = work.tile([H, 1], F32, tag="s2")
            nc.vector.reduce_sum(out=s2[:, :], in_=u2[:, :], axis=AX)
            bound = work.tile([H, 1], F32, tag="bound")
            nc.scalar.activation(
                out=bound[:, :], in_=s2[:, :], func=AF.Sqrt, bias=0.0, scale=64.0
            )
            eb = work.tile([H, 1], F32, tag="eb")
            nc.vector.tensor_sub(out=eb[:, :], in0=e_sb[:, :], in1=bound[:, :])

            # ---- cls score / attention prologue ------------------------
            a_sb = singles.tile([H, 1025], BF16, tag="asb")
            aT = singles.tile([128, H * NK], BF16, tag="aT")
            se = work.tile([H, 3], F32, tag="seall")
            se0 = se[:, 2:3]
            psS0 = psm.tile([H, 1], F32, tag="psm")
            for c in range(NK):
                nc.tensor.matmul(
                    psS0[:, :], lhsT=ugT[:, H * c : H * (c + 1)], rhs=zn0_col[:, c : c + 1],
                    start=(c == 0), stop=(c == NK - 1),
                )
            nc.scalar.activation(
                out=a_sb[:, 0:1], in_=psS0[:, :], func=AF.Exp,
                bias=eb[:, 0:1], scale=1.0, accum_out=se0,
            )
            a0p = pt.tile([128, 16], BF16, tag="pt")
            nc.tensor.transpose(out=a0p[0:1, 0:H], in_=a_sb[:, 0:1], identity=ident[0:H, 0:H])
            aT0 = singles.tile([1, H], BF16, tag="aT0")
            nc.scalar.copy(out=aT0[:, :], in_=a0p[0:1, 0:H])
            psM = pbig.tile([H, D], F32, tag="pbig")
            for half in range(2):
                nc.tensor.matmul(
                    psM[:, 512 * half : 512 * (half + 1)], lhsT=aT0[:, :],
                    rhs=zn0_row[:, 512 * half : 512 * (half + 1)],
                    start=True, stop=False, skip_group_check=True,
                )

            # ---- x LayerNorm pipeline: stats -> normalize -> transpose --
            for k in range(NK):
                xk = xks[k]
                stats = work.tile([128, 2, 6], F32, tag="stats", name=f"stats{k}")
                nc.vector.bn_stats(out=stats[:, 0, :], in_=xk[:, 0:512])
                nc.vector.bn_stats(out=stats[:, 1, :], in_=xk[:, 512:1024])
                mv = work.tile([128, 2], F32, tag="mv", name=f"mv{k}")
                nc.vector.bn_aggr(out=mv[:, :], in_=stats[:, :, :])
                nc.scalar.activation(
                    out=mv[:, 1:2], in_=mv[:, 1:2], func=AF.Sqrt,
                    bias=eps_t[:, :], scale=1.0,
                )
                rs = work.tile([128, 1], F32, tag="rs", name=f"rs{k}")
                nc.vector.tensor_copy(out=rs[:, :], in_=mv[:, 1:2])
                nc.vector.reciprocal(out=rs[:, :], in_=rs[:, :])
                nc.vector.tensor_scalar(
                    out=znorm_all[:, D * k : D * (k + 1)], in0=xk[:, :],
                    scalar1=mv[:, 0:1], scalar2=rs[:, 0:1],
                    op0=OP.subtract, op1=OP.mult,
                )
                eng = nc.sync if (k % 2 == 0) else nc.scalar
                eng.dma_start_transpose(
                    out=znT3[:, :, 128 * k : 128 * (k + 1)],
                    in_=znorm_all[:, D * k : D * (k + 1)],
                )

            # ---- scores/softmax per 512-col half, then Mrow ------------
            for half in range(2):
                psS = psm.tile([H, 512], F32, tag="psm", name=f"psS{half}")
                for c in range(NK):
                    nc.tensor.matmul(
                        psS[:, :], lhsT=ugT[:, H * c : H * (c + 1)],
                        rhs=znT_all[:, D * c + 512 * half : D * c + 512 * (half + 1)],
                        start=(c == 0), stop=(c == NK - 1),
                    )
                nc.scalar.activation(
                    out=a_sb[:, 1 + 512 * half : 1 + 512 * (half + 1)], in_=psS[:, :],
                    func=AF.Exp, bias=eb[:, 0:1], scale=1.0,
                    accum_out=se[:, half : half + 1],
                )
                for kk in range(4):
                    k = 4 * half + kk
                    atpk = pt.tile([128, 16], BF16, tag="pt", name=f"atp{k}")
                    nc.tensor.transpose(
                        out=atpk[:, 0:H],
                        in_=a_sb[:, 1 + 128 * k : 1 + 128 * (k + 1)],
                        identity=ident[0:H, 0:H],
                    )
                    nc.scalar.copy(out=aT[:, H * k : H * (k + 1)], in_=atpk[:, 0:H])
                    for dh in range(2):
                        nc.tensor.matmul(
                            psM[:, 512 * dh : 512 * (dh + 1)], lhsT=aT[:, H * k : H * (k + 1)],
                            rhs=znorm_all[:, D * k + 512 * dh : D * k + 512 * (dh + 1)],
                            start=False, stop=(k == NK - 1), skip_group_check=True,
                        )

            rinv = work.tile([H, 1], F32, tag="rinv")
            nc.vector.reduce_sum(out=rinv[:, :], in_=se[:, :], axis=AX)
            nc.vector.reciprocal(out=rinv[:, :], in_=rinv[:, :])

            # mrow = psM * rinv * gamma  (beta added after transpose)
            mrow = singles.tile([H, D], BF16, tag="mrow")
            nc.vector.scalar_tensor_tensor(
                out=mrow[:, :], in0=psM[:, :], scalar=rinv[:, 0:1], in1=gam16[:, :],
                op0=OP.mult, op1=OP.mult,
            )
            mT = singles.tile([128, H * NK], BF16, tag="mT")
            for c in range(NK):
                mtp = pt.tile([128, 16], BF16, tag="pt", name=f"mtp{c}")
                nc.tensor.transpose(
                    out=mtp[:, 0:H], in_=mrow[:, 128 * c : 128 * (c + 1)],
                    identity=ident[0:H, 0:H],
                )
                # fold the PSUM->SBUF copy into the beta add
                nc.vector.tensor_scalar_add(
                    out=mT[:, H * c : H * (c + 1)], in0=mtp[:, 0:H],
                    scalar1=bet_col[:, c : c + 1],
                )

            # ---- ctx via [16,512] trick + transpose-select --------------
            # ps16[h, n] = sum_d Mln[h, d] * w_v[n, d]; ctx[n] = ps16[n//64, n]
            ps16 = pbig.tile([H, D], F32, tag="pbig")
            for half in range(2):
                for c in range(NK):
                    nc.tensor.matmul(
                        ps16[:, 512 * half : 512 * (half + 1)], lhsT=mT[:, H * c : H * (c + 1)],
                        rhs=wv_all[:, D * c + 512 * half : D * c + 512 * (half + 1)],
                        start=(c == 0), stop=(c == NK - 1),
                        skip_group_check=True,
                    )
            c16 = singles.tile([H, D], BF16, tag="c16")
            nc.scalar.copy(out=c16[:, :], in_=ps16[:, :])
            ctx_bf = singles.tile([128, NK], BF16, tag="ctxbf")
            psO = pbig.tile([1, D], F32, tag="pbig")
            for j in range(NK):
                ctp = pt.tile([128, 16], BF16, tag="pt", name=f"ctp{j}")
                nc.tensor.transpose(
                    out=ctp[:, 0:H], in_=c16[:, 128 * j : 128 * (j + 1)],
                    identity=ident[0:H, 0:H],
                )
                nc.vector.scalar_tensor_tensor(
                    out=ctx_bf[0:64, j : j + 1], in0=ctp[0:64, 2 * j : 2 * j + 1],
                    scalar=1.0, in1=bv_col[0:64, j : j + 1], op0=OP.mult, op1=OP.add,
                )
                nc.vector.scalar_tensor_tensor(
                    out=ctx_bf[64:128, j : j + 1], in0=ctp[64:128, 2 * j + 1 : 2 * j + 2],
                    scalar=1.0, in1=bv_col[64:128, j : j + 1], op0=OP.mult, op1=OP.add,
                )
            for a in range(NK):
                for half in range(2):
                    nc.tensor.matmul(
                        psO[:, 512 * half : 512 * (half + 1)],
                        lhsT=ctx_bf[:, a : a + 1],
                        rhs=wo_all[:, D * a + 512 * half : D * a + 512 * (half + 1)],
                        start=(a == 0), stop=False,
                        skip_group_check=True,
                    )
            for half in range(2):
                nc.tensor.matmul(
                    psO[:, 512 * half : 512 * (half + 1)], lhsT=ident[0:1, 0:1],
                    rhs=bo_row[0:1, 512 * half : 512 * (half + 1)],
                    start=False, stop=True, skip_group_check=True,
                )
            out_sb = singles.tile([1, D], F32, tag="outsb")
            nc.scalar.copy(out=out_sb[:, :], in_=psO[:, :])
            nc.sync.dma_start(out=out_e[:, :], in_=out_sb[:, :])

    nc.compile()
    return nc


def _pack128(a):
    # [1024, 1024] -> [128, 8*1024] with out[p, k*1024+i] = a[128k+p, i]
    return np.ascontiguousarray(
        a.reshape(NK, 128, D).transpose(1, 0, 2).reshape(128, NK * D)
    )


def _col(a):
    # [1024] -> [128, 8] with out[p, c] = a[128c+p]
    return np.ascontiguousarray(a.reshape(NK, 128).T)


def _prep_in_maps(inputs):
    bf = ml_dtypes.bfloat16
    f32 = np.float32

    def c(a, dt):
        return np.asarray(a, dtype=dt)

    x = c(inputs["x"], bf)
    smalls = np.concatenate(
        [
            _col(c(inputs["gamma"], f32)),
            _col(c(inputs["beta"], f32)),
            _col(c(inputs["b_q"], f32)),
            _col(c(inputs["b_v"], f32)),
        ],
        axis=1,
    )
    shared = {
        "cls": np.ascontiguousarray(c(inputs["cls_token"], f32)),
        "gam": np.ascontiguousarray(c(inputs["gamma"], f32)),
        "bet": np.ascontiguousarray(c(inputs["beta"], f32)),
        "smalls": np.ascontiguousarray(smalls),
        "bkc": _col(c(inputs["b_k"], bf)),
        "bo": np.ascontiguousarray(c(inputs["b_o"], bf)),
        "wqp": _pack128(c(np.asarray(inputs["w_q"]).T, bf)),
        "wkp": _pack128(c(inputs["w_k"], bf)),
        "wvp": _pack128(c(np.asarray(inputs["w_v"]).T, bf)),
        "wop": _pack128(c(np.asarray(inputs["w_o"]).T, bf)),
    }
    return [{"x": _pack128(x[b]), **shared} for b in range(8)]


def run(inputs, trace=False, **kw):
    if "nc" not in _CACHE:
        _CACHE["nc"] = _build()
    nc = _CACHE["nc"]
    in_maps = _prep_in_maps(inputs)
    res = run_bass_kernel_spmd(nc, in_maps, core_ids=list(range(8)), trace=trace, **kw)
    out = np.stack([np.asarray(res.results[b]["out"][0], dtype=np.float32) for b in range(8)])
    return out, res


def kernel(**inputs):
    out, _ = run(inputs, trace=False)
    return out
